# revision 50
# baseline (speedup 1.0000x reference)
"""Causal multi-head attention on 8 trn2 NeuronCores.

Problem: B=2, T=2048, C=1024, H=16 heads, D=64, fp32.
    q/k/v = x @ W{q,k,v}.T ; causal softmax(q k^T / sqrt(D)) @ v ; out @ Wo.T

Sharding (Megatron-style): data-parallel over batch (2 groups of 4 cores),
tensor-parallel over heads within a group (4 heads per core; Wq/Wk/Wv
column-sharded, Wo row-sharded). Each core emits a partial y[b].T; the host
sums the 4 partials per batch and transposes back.

Per-core device program (bf16 matmuls, f32 PSUM accumulation; rel-err vs
the f32 reference ~4e-3 on hardware). Engine budget per core ~ PE 127us,
ACT (exp) 90us, DVE ~77us, Pool (SBUF-only: broadcasts) ~14us; the
emission order keeps PE saturated and reserves ACT almost solely for exp:

  - warmup: large input DMAs stream in (wq, xT kc=0, wk, xT kc=1..7, wv,
    wo) while qk pair0 (heads 0,1) runs kc-outer into 8 PSUM banks, so
    each xT chunk is consumed as its DMA lands.
  - attention runs n-major in fused tq-chunk pass-pairs (0,1) / (2,3):
    per tk chunk i one scores matmul pair (split only at the PSUM bank
    boundary) and ONE wide exp cover both chunks; AV accumulates into a
    per-chunk [65,512] ctx PSUM tile with a lag-1 software pipeline, and
    each chunk is normalized (ones-column denominator) when it completes.
  - the diagonal scores/exp are trimmed to start at column i*128; the
    sub-diagonal remainder is memset + causal-masked (multiplicative).
  - PE filler units flow through a shared deque, one per iteration, so
    PE never waits on ACT: the v-projection pipeline fills head 0, qk
    pair1 fills head 1 (+ head 2's first pair), and the output
    projection (jc-paired matmuls + bf16 DMA out) streams per 512-col
    chunk as soon as all four heads' ctx for that chunk is normalized —
    heads 2/3 are pair-interleaved so h3's early y units fill h2.

Softmax denominator trick: v is augmented with a pad-valued ones column
per head, so ctx PSUM row 64 accumulates sum(exp). No max-subtraction:
scores/8 ~ N(0,1), exp never overflows and matches softmax exactly.
"""

import os

import numpy as np

B, T, C, H, D = 2, 2048, 1024, 16, 64
NCORES = 8
GROUPS = 4          # tensor-parallel groups per batch
HPC = H // GROUPS   # heads per core = 4
J = HPC * D         # per-core projection width = 256
P = 128
NT = T // P         # 16 row chunks
KC = C // P         # 8 contraction chunks
NQ = T // 512       # 4 query 512-chunks
E1 = D + 1          # 65: head dim + ones column

MM_DTYPE = os.environ.get("MM_DTYPE", "bfloat16")  # "float32r" or "bfloat16"
OUT_DT = os.environ.get("OUT_DT", "bfloat16")  # yT partial dtype
_COMPILED = None


def build_program(dtype_mm=None, variant="full", mult=1):
    """Emit the SPMD bass program (same on all 8 cores)."""
    import concourse.bass as bass
    import concourse.mybir as mybir
    import concourse.tile as tile
    from concourse import bacc
    from concourse.masks import make_upper_triangular

    dtype_mm = dtype_mm or MM_DTYPE
    f32 = mybir.dt.float32
    bf16 = mybir.dt.bfloat16
    md = getattr(mybir.dt, dtype_mm)
    bf16_in = dtype_mm == "bfloat16"
    av_dt = bf16  # exp output + v path (ACT cannot write float32r)
    in_dt = md if bf16_in else f32
    out_dt = getattr(mybir.dt, OUT_DT)

    def ld(ap):  # DRAM-side view for DMA into an md-typed tile
        return ap if bf16_in else ap.bitcast(md)

    nc = bacc.Bacc("TRN2", target_bir_lowering=False, debug=False)

    xT = nc.dram_tensor("xT", [C, T], in_dt, kind="ExternalInput").ap()
    wq = nc.dram_tensor("wq_t", [C, J], in_dt, kind="ExternalInput").ap()
    wk = nc.dram_tensor("wk_t", [C, J], in_dt, kind="ExternalInput").ap()
    wv = nc.dram_tensor("wv_t", [C, J], in_dt, kind="ExternalInput").ap()
    wo = nc.dram_tensor("wo_t", [J, C], in_dt, kind="ExternalInput").ap()
    pad = nc.dram_tensor("pad", [T, 1], f32, kind="ExternalInput").ap()
    yT = nc.dram_tensor("yT", [C, T], out_dt, kind="ExternalOutput").ap()

    with tile.TileContext(nc) as tc:
      with tc.tile_pool(name="const", bufs=1) as const_pool:
        # mask[tk, tq] = 1.0 iff tk <= tq (causal keep-region, multiplicative)
        mask = const_pool.tile([P, P], av_dt)
        make_upper_triangular(nc, mask, val=1.0, diag=True)
        pad_sb = const_pool.tile([P, NT], f32)

        with tc.tile_pool(name="sb", bufs=1) as sb:
          for _rep in range(mult):
            # ---- input DMA: few large transfers (dispatch-latency, not
            # bandwidth, limits the stream); wq/wk/xT first, they gate qk0
            w_tiles = {
                name: sb.tile([P, KC, J], md, tag=f"w{name}", name=f"w{name}_sb")
                for name in ("q", "k", "v")
            }
            xT_sb = sb.tile([P, KC, T], md, tag="xT")
            wo_sb = sb.tile([P, 2, C], md, tag="wo")
            nc.sync.dma_start(
                out=w_tiles["q"], in_=ld(wq).rearrange("(kc p) j -> p kc j", p=P)
            )
            nc.sync.dma_start(
                out=xT_sb[:, 0, :], in_=ld(xT[0:P, :])
            )
            nc.sync.dma_start(
                out=w_tiles["k"], in_=ld(wk).rearrange("(kc p) j -> p kc j", p=P)
            )
            pad_v = pad.rearrange("(i p) one -> p (i one)", p=P)
            nc.sync.dma_start(out=pad_sb, in_=pad_v)
            for kc in range(1, KC):  # per-chunk so qk0 streams with arrival
                nc.sync.dma_start(
                    out=xT_sb[:, kc, :], in_=ld(xT[kc * P : (kc + 1) * P, :])
                )
            nc.sync.dma_start(
                out=w_tiles["v"], in_=ld(wv).rearrange("(kc p) j -> p kc j", p=P)
            )
            nc.sync.dma_start(
                out=wo_sb, in_=ld(wo).rearrange("(jc p) c -> p jc c", p=P)
            )

            qT_sb = sb.tile([P, 2, T], md, tag="qT")
            kT_sb = sb.tile([P, 2, T], md, tag="kT")
            v1_sb = sb.tile([P, NT, HPC * E1], av_dt, tag="v1")
            ctxn_sb = sb.tile([P, 2, T], md, tag="ctxn")

            with (
                tc.tile_pool(name="e_sb", bufs=3) as e_pool,
                tc.tile_pool(name="norm", bufs=2) as norm_pool,
                tc.tile_pool(name="y_sb", bufs=4) as y_sb_pool,
            ):
                # ---------- filler units (one unit ~ 0.5-2us of PE) ----------
                def emit_qk_chunk(pair, name, n, ps_pool):
                    """q or k chunk n of a head pair -> qT/kT_sb[:, pair, n]."""
                    dst = qT_sb if name == "q" else kT_sb
                    ps = ps_pool.tile([P, 512], f32, tag="qk", name="qk_ps_t")
                    for kc in range(KC):
                        nc.tensor.matmul(
                            ps,
                            lhsT=w_tiles[name][:, kc, pair * P : (pair + 1) * P],
                            rhs=xT_sb[:, kc, n * 512 : (n + 1) * 512],
                            start=(kc == 0),
                            stop=(kc == KC - 1),
                        )
                    # GPSIMD cannot read PSUM: drains go DVE/ACT only
                    eng = nc.vector.tensor_copy if n % 2 else nc.scalar.copy
                    eng(dst[:, pair, n * 512 : (n + 1) * 512], ps)

                def emit_v(i, v_ps):
                    """v chunk i: [t=128, J] psum -> pad-scale -> v1_sb + col."""
                    ps = v_ps.tile([P, J], f32, tag="v", name="v_ps_t")
                    for kc in range(KC):
                        nc.tensor.matmul(
                            ps,
                            lhsT=xT_sb[:, kc, i * P : (i + 1) * P],
                            rhs=w_tiles["v"][:, kc, :],
                            start=(kc == 0),
                            stop=(kc == KC - 1),
                        )
                    nc.vector.tensor_scalar_mul(ps, ps, pad_sb[:, i : i + 1])
                    v1_i = v1_sb[:, i, :]
                    for h in range(HPC):
                        eng = nc.scalar.copy if h == 0 else nc.vector.tensor_copy
                        eng(v1_i[:, h * E1 : h * E1 + D], ps[:, h * D : (h + 1) * D])
                    ones_v = v1_i.rearrange("p (h e) -> p h e", e=E1)[:, :, D : D + 1]
                    pc = pad_sb[:, i : i + 1]
                    pad_b = bass.AP(
                        tensor=pc.tensor,
                        offset=pc.offset,
                        ap=[pc.ap[0], [0, HPC], [0, 1]],
                    )
                    nc.vector.tensor_copy(ones_v, pad_b)

                def emit_y_pair(n, oc0, y_ps, last):
                    """final y chunk n, oc pair (oc0, oc0+1): jc-accumulated
                    matmuls + drain + DMA out."""
                    cols = slice(n * 512, (n + 1) * 512)
                    yo = y_sb_pool.tile(
                        [P, 2, 512], out_dt, tag="yo", name="yo_t", bufs=4
                    )
                    for u, oc in enumerate((oc0, oc0 + 1)):
                        ps = y_ps.tile([P, 512], f32, tag="y", name="y_ps_t")
                        for jc in range(2):
                            nc.tensor.matmul(
                                ps,
                                lhsT=wo_sb[:, jc, oc * P : (oc + 1) * P],
                                rhs=ctxn_sb[:, jc, cols],
                                start=(jc == 0),
                                stop=(jc == 1),
                            )
                        eng = nc.scalar.copy if u else nc.vector.tensor_copy
                        eng(yo[:, u, :], ps)
                    out_v = yT[oc0 * P : (oc0 + 2) * P, cols].rearrange(
                        "(two p) t -> p two t", p=P
                    )
                    nc.sync.dma_start(out=out_v, in_=yo)

                # ---------- n-major attention, one fused pass-pair ----------
                from collections import deque

                fill_q = deque()  # PE filler units, popped one per iteration

                def emit_pair(h, na, fillers=(), y_stream=None):
                    """Head h, fused tq-chunk pair (na, na+1).

                    One [P,1024]-wide scores matmul + exp per tk chunk covers
                    both tq chunks (halves ACT's per-call overhead); each
                    chunk keeps its own [65,512] ctx accumulator, normalized
                    (and y-streamed) the moment it completes.
                    """
                    jc, poff = h // 2, (h % 2) * D
                    qTh = qT_sb[poff : poff + D, jc, :]
                    kTh = kT_sb[poff : poff + D, jc, :]
                    v1h = lambda i: v1_sb[:, i, h * E1 : (h + 1) * E1]
                    nb = na + 1
                    base = na * 512
                    ni = 4 * nb + 4
                    e = e_pool.tile([P, NT, 1024], av_dt, tag="e", name="e_t")
                    ctxs = [
                        ctx_ps.tile(
                            [E1, 512], f32, tag=f"ctx{c}", name="ctx_t", bufs=1
                        )
                        for c in range(2)
                    ]

                    def finish_chunk(c):
                        """normalize chunk na+c; queue its y units."""
                        n = na + c
                        cols = slice(n * 512, (n + 1) * 512)
                        rec = norm_pool.tile([1, 512], f32, tag="rec", name="rec_t")
                        bc = norm_pool.tile([D, 512], f32, tag="bc", name="bc_t")
                        nc.vector.reciprocal(rec, ctxs[c][D : D + 1, :])
                        nc.gpsimd.partition_broadcast(bc, rec)
                        nc.vector.tensor_mul(
                            ctxn_sb[poff : poff + D, jc, cols], ctxs[c][0:D, :], bc
                        )
                        if y_stream:
                            y_stream(n, fill_q)

                    def emit_av(i):
                        for c, n in ((0, na), (1, nb)):
                            if i <= 4 * n + 3:
                                nc.tensor.matmul(
                                    ctxs[c],
                                    lhsT=v1h(i),
                                    rhs=e[:, i, c * 512 : (c + 1) * 512],
                                    start=(i == 0),
                                    stop=(i == 4 * n + 3),
                                )
                        if i == 4 * na + 3:
                            finish_chunk(0)

                    fill_q.extend(fillers)
                    pending = None
                    for i in range(ni):
                        lo = max(i * P, base)  # diagonal trim
                        off = lo - base
                        s = sc_ps.tile([P, 1024], f32, tag="s", name="s_t")
                        # a matmul output cannot span two PSUM banks: split at
                        # the 512-col boundary; exp still reads the full width
                        for b0, b1 in ((off, 512), (max(off, 512), 1024)):
                            if b0 < b1:
                                nc.tensor.matmul(
                                    s[:, b0:b1],
                                    lhsT=kTh[:, i * P : (i + 1) * P],
                                    rhs=qTh[:, base + b0 : base + b1],
                                    start=True,
                                    stop=True,
                                )
                        nc.scalar.activation(
                            e[:, i, off:1024],
                            s[:, off:1024],
                            mybir.ActivationFunctionType.Exp,
                            scale=0.125,  # 1/sqrt(D)
                        )
                        nd = i // 4
                        if nd in (na, nb):  # diagonal block
                            zlo = (nd - na) * 512
                            if off > zlo:
                                nc.vector.memset(e[:, i, zlo:off], 0.0)
                            nc.vector.tensor_mul(
                                e[:, i, off : off + P],
                                e[:, i, off : off + P],
                                mask,
                            )
                        if pending is not None:
                            emit_av(pending)
                        if fill_q:
                            fill_q.popleft()()
                        pending = i
                    emit_av(pending)
                    finish_chunk(1)

                # ---------- schedule ----------
                # warmup: qk pair0 kc-outer over the full width (8 PSUM
                # banks) so every xT chunk is consumed as its DMA lands
                sc_ps = ctx_ps = None
                with tc.tile_pool(name="qk0_ps", bufs=1, space="PSUM") as qk0_ps:
                    pss = {
                        name: qk0_ps.tile(
                            [P, T], f32, tag=f"qk0{name}", name=f"ps0{name}"
                        )
                        for name in ("q", "k")
                    }
                    for kc in range(KC):
                        for name in ("q", "k"):
                            for n in range(NQ):
                                nc.tensor.matmul(
                                    pss[name][:, n * 512 : (n + 1) * 512],
                                    lhsT=w_tiles[name][:, kc, 0:P],
                                    rhs=xT_sb[:, kc, n * 512 : (n + 1) * 512],
                                    start=(kc == 0),
                                    stop=(kc == KC - 1),
                                )
                    # chunk-major drain order: pair-0 scores need (q0,k0) first
                    for n in range(NQ):
                        for name, dst in (("q", qT_sb), ("k", kT_sb)):
                            # ACT is idle pre-attention; keep DVE free for v
                            eng = nc.vector.tensor_copy if n % 2 else nc.scalar.copy
                            eng(
                                dst[:, 0, n * 512 : (n + 1) * 512],
                                pss[name][:, n * 512 : (n + 1) * 512],
                            )
                with (
                    tc.tile_pool(name="sc_ps", bufs=2, space="PSUM") as sc_ps,
                    tc.tile_pool(name="ctx_ps", bufs=2, space="PSUM") as ctx_ps,
                ):
                    with tc.tile_pool(name="v_ps", bufs=2, space="PSUM") as v_ps:
                        # v(i) streams as filler ahead of its AV use so pair-0
                        # scores (and ACT's exp) start as early as possible
                        emit_v(0, v_ps)
                        emit_v(1, v_ps)
                        vu = [lambda ii=i: emit_v(ii, v_ps) for i in range(2, 16)]
                        emit_pair(0, 0, vu[0:8])
                        emit_pair(0, 2, vu[8:14])
                    with tc.tile_pool(name="qk1_ps", bufs=2, space="PSUM") as qk1_ps:
                        # qk pair1: chunks 0,1 must finish in h1 (h2's and
                        # h3's first pairs span them); 2,3 fill h2's pair 0
                        qu = [
                            lambda nm=nm, nn=n1: emit_qk_chunk(1, nm, nn, qk1_ps)
                            for n1 in range(NQ)
                            for nm in ("q", "k")
                        ]
                        emit_pair(1, 0, qu[0:2])
                        emit_pair(1, 2, qu[2:4])
                        emit_pair(2, 0, qu[4:8])
                    with tc.tile_pool(name="y_ps", bufs=2, space="PSUM") as y_ps:

                        def y_stream(n, q):
                            last = n == NQ - 1
                            for oc0 in range(0, KC, 2):
                                q.append(
                                    lambda nn=n, oo=oc0, ll=last: emit_y_pair(
                                        nn, oo, y_ps, ll
                                    )
                                )

                        # interleave h2/h3 pairs: h3's early y units become
                        # PE filler for h2's second pair
                        emit_pair(3, 0, (), y_stream)
                        emit_pair(2, 2)
                        emit_pair(3, 2, (), y_stream)
                        while fill_q:
                            fill_q.popleft()()

    nc.compile()
    return nc


def make_in_maps(x, pad_mask, Wq, Wk, Wv, Wo):
    """Host-side sharding: per-core input dict."""
    if MM_DTYPE == "bfloat16":
        import ml_dtypes

        in_np = ml_dtypes.bfloat16
    else:
        in_np = np.float32
    x = np.asarray(x, dtype=np.float32)
    pad_f = np.asarray(pad_mask).astype(np.float32).reshape(B, T, 1)
    # cast once, slice per core (cheaper than per-core casting)
    xT_b = [np.ascontiguousarray(x[b].T).astype(in_np) for b in range(B)]
    WqT, WkT, WvT = (
        np.asarray(w, dtype=np.float32).T.astype(in_np) for w in (Wq, Wk, Wv)
    )
    Wo_c = np.asarray(Wo, dtype=np.float32).astype(in_np)
    in_maps = []
    for c in range(NCORES):
        b, g = c // GROUPS, c % GROUPS
        jr = slice(g * J, (g + 1) * J)
        in_maps.append(
            {
                "xT": xT_b[b],
                "wq_t": np.ascontiguousarray(WqT[:, jr]),
                "wk_t": np.ascontiguousarray(WkT[:, jr]),
                "wv_t": np.ascontiguousarray(WvT[:, jr]),
                "wo_t": np.ascontiguousarray(Wo_c[:, jr].T),
                "pad": np.ascontiguousarray(pad_f[b]),
            }
        )
    return in_maps


def unshard(results):
    """Sum the 4 tensor-parallel partials per batch; transpose back."""
    y = np.empty((B, T, C), dtype=np.float32)
    for b in range(B):
        acc = results[b * GROUPS]["yT"].astype(np.float32)
        for g in range(1, GROUPS):
            acc = acc + results[b * GROUPS + g]["yT"].astype(np.float32)
        y[b] = acc.T
    return y


def kernel(x, pad_mask, Wq, Wk, Wv, Wo):
    global _COMPILED
    from concourse.bass_utils import run_bass_kernel_spmd

    if _COMPILED is None:
        _COMPILED = build_program()
    in_maps = make_in_maps(x, pad_mask, Wq, Wk, Wv, Wo)
    res = run_bass_kernel_spmd(_COMPILED, in_maps, core_ids=list(range(NCORES)))
    return unshard(res.results)


# revision 55
# speedup vs baseline: 1.0229x; 1.0229x over previous
"""Causal multi-head attention on 8 trn2 NeuronCores.

Problem: B=2, T=2048, C=1024, H=16 heads, D=64, fp32.
    q/k/v = x @ W{q,k,v}.T ; causal softmax(q k^T / sqrt(D)) @ v ; out @ Wo.T

Sharding (Megatron-style): data-parallel over batch (2 groups of 4 cores),
tensor-parallel over heads within a group (4 heads per core; Wq/Wk/Wv
column-sharded, Wo row-sharded). Each core emits a partial y[b].T; the host
sums the 4 partials per batch and transposes back.

Per-core device program (bf16 matmuls, f32 PSUM accumulation; rel-err vs
the f32 reference ~4e-3 on hardware). Engine budget per core ~ PE 127us,
ACT (exp) 90us, DVE ~77us, Pool (SBUF-only: broadcasts) ~14us; the
emission order keeps PE saturated and reserves ACT almost solely for exp:

  - warmup: large input DMAs stream in (wq, xT kc=0, wk, xT kc=1..7, wv,
    wo) while qk pair0 (heads 0,1) runs kc-outer into 8 PSUM banks, so
    each xT chunk is consumed as its DMA lands.
  - attention runs n-major in fused tq-chunk pass-pairs (0,1) / (2,3):
    per tk chunk i one scores matmul pair (split only at the PSUM bank
    boundary) and ONE wide exp cover both chunks; AV accumulates into a
    per-chunk [65,512] ctx PSUM tile with a lag-1 software pipeline, and
    each chunk is normalized (ones-column denominator) when it completes.
  - the diagonal scores/exp are trimmed to start at column i*128; the
    sub-diagonal remainder is memset + causal-masked (multiplicative).
  - PE filler units flow through a shared deque, one per iteration, so
    PE never waits on ACT: the v-projection pipeline fills head 0, qk
    pair1 fills head 1 (+ head 2's first pair), and the output
    projection (jc-paired matmuls + bf16 DMA out) streams per 512-col
    chunk as soon as all four heads' ctx for that chunk is normalized —
    heads 2/3 are pair-interleaved so h3's early y units fill h2.

Softmax denominator trick: v is augmented with a pad-valued ones column
per head, so ctx PSUM row 64 accumulates sum(exp). No max-subtraction:
scores/8 ~ N(0,1), exp never overflows and matches softmax exactly.
"""

import os

import numpy as np

B, T, C, H, D = 2, 2048, 1024, 16, 64
NCORES = 8
GROUPS = 4          # tensor-parallel groups per batch
HPC = H // GROUPS   # heads per core = 4
J = HPC * D         # per-core projection width = 256
P = 128
NT = T // P         # 16 row chunks
KC = C // P         # 8 contraction chunks
NQ = T // 512       # 4 query 512-chunks
E1 = D + 1          # 65: head dim + ones column

MM_DTYPE = os.environ.get("MM_DTYPE", "bfloat16")  # "float32r" or "bfloat16"
OUT_DT = os.environ.get("OUT_DT", "bfloat16")  # yT partial dtype
_COMPILED = None


def build_program(dtype_mm=None, variant="full", mult=1):
    """Emit the SPMD bass program (same on all 8 cores)."""
    import concourse.bass as bass
    import concourse.mybir as mybir
    import concourse.tile as tile
    from concourse import bacc
    from concourse.masks import make_upper_triangular

    dtype_mm = dtype_mm or MM_DTYPE
    f32 = mybir.dt.float32
    bf16 = mybir.dt.bfloat16
    md = getattr(mybir.dt, dtype_mm)
    bf16_in = dtype_mm == "bfloat16"
    av_dt = bf16  # exp output + v path (ACT cannot write float32r)
    in_dt = md if bf16_in else f32
    out_dt = getattr(mybir.dt, OUT_DT)

    def ld(ap):  # DRAM-side view for DMA into an md-typed tile
        return ap if bf16_in else ap.bitcast(md)

    nc = bacc.Bacc("TRN2", target_bir_lowering=False, debug=False)

    xT = nc.dram_tensor("xT", [C, T], in_dt, kind="ExternalInput").ap()
    wq = nc.dram_tensor("wq_t", [C, J], in_dt, kind="ExternalInput").ap()
    wk = nc.dram_tensor("wk_t", [C, J], in_dt, kind="ExternalInput").ap()
    wv = nc.dram_tensor("wv_t", [C, J], in_dt, kind="ExternalInput").ap()
    wo = nc.dram_tensor("wo_t", [J, C], in_dt, kind="ExternalInput").ap()
    pad = nc.dram_tensor("pad", [T, 1], f32, kind="ExternalInput").ap()
    yT = nc.dram_tensor("yT", [C, T], out_dt, kind="ExternalOutput").ap()

    with tile.TileContext(nc) as tc:
      with tc.tile_pool(name="const", bufs=1) as const_pool:
        # mask[tk, tq] = 1.0 iff tk <= tq (causal keep-region, multiplicative)
        mask = const_pool.tile([P, P], av_dt)
        make_upper_triangular(nc, mask, val=1.0, diag=True)
        pad_sb = const_pool.tile([P, NT], f32)

        with tc.tile_pool(name="sb", bufs=1) as sb:
          for _rep in range(mult):
            # ---- input DMA: few large transfers (dispatch-latency, not
            # bandwidth, limits the stream); wq/wk/xT first, they gate qk0
            w_tiles = {
                name: sb.tile([P, KC, J], md, tag=f"w{name}", name=f"w{name}_sb")
                for name in ("q", "k", "v")
            }
            xT_sb = sb.tile([P, KC, T], md, tag="xT")
            wo_sb = sb.tile([P, 2, C], md, tag="wo")
            nc.sync.dma_start(
                out=w_tiles["q"], in_=ld(wq).rearrange("(kc p) j -> p kc j", p=P)
            )
            nc.sync.dma_start(
                out=xT_sb[:, 0, :], in_=ld(xT[0:P, :])
            )
            nc.sync.dma_start(
                out=w_tiles["k"], in_=ld(wk).rearrange("(kc p) j -> p kc j", p=P)
            )
            pad_v = pad.rearrange("(i p) one -> p (i one)", p=P)
            nc.sync.dma_start(out=pad_sb, in_=pad_v)
            for kc in range(1, KC):  # per-chunk so qk0 streams with arrival
                nc.sync.dma_start(
                    out=xT_sb[:, kc, :], in_=ld(xT[kc * P : (kc + 1) * P, :])
                )
            nc.sync.dma_start(
                out=w_tiles["v"], in_=ld(wv).rearrange("(kc p) j -> p kc j", p=P)
            )
            nc.sync.dma_start(
                out=wo_sb, in_=ld(wo).rearrange("(jc p) c -> p jc c", p=P)
            )

            qT_sb = sb.tile([P, 2, T], md, tag="qT")
            kT_sb = sb.tile([P, 2, T], md, tag="kT")
            v1_sb = sb.tile([P, NT, HPC * E1], av_dt, tag="v1")
            ctxn_sb = sb.tile([P, 2, T], md, tag="ctxn")

            with (
                tc.tile_pool(name="e_sb", bufs=3) as e_pool,
                tc.tile_pool(name="norm", bufs=2) as norm_pool,
                tc.tile_pool(name="y_sb", bufs=4) as y_sb_pool,
            ):
                # ---------- filler units (one unit ~ 0.5-2us of PE) ----------
                def emit_qk_chunk(pair, name, n, ps_pool):
                    """q or k chunk n of a head pair -> qT/kT_sb[:, pair, n]."""
                    dst = qT_sb if name == "q" else kT_sb
                    ps = ps_pool.tile([P, 512], f32, tag="qk", name="qk_ps_t")
                    for kc in range(KC):
                        nc.tensor.matmul(
                            ps,
                            lhsT=w_tiles[name][:, kc, pair * P : (pair + 1) * P],
                            rhs=xT_sb[:, kc, n * 512 : (n + 1) * 512],
                            start=(kc == 0),
                            stop=(kc == KC - 1),
                        )
                    # GPSIMD cannot read PSUM: drains go DVE/ACT only
                    eng = nc.vector.tensor_copy if n % 2 else nc.scalar.copy
                    eng(dst[:, pair, n * 512 : (n + 1) * 512], ps)

                def emit_v(i, v_ps):
                    """v chunk i: [t=128, J] psum -> pad-scale -> v1_sb + col."""
                    ps = v_ps.tile([P, J], f32, tag="v", name="v_ps_t")
                    for kc in range(KC):
                        nc.tensor.matmul(
                            ps,
                            lhsT=xT_sb[:, kc, i * P : (i + 1) * P],
                            rhs=w_tiles["v"][:, kc, :],
                            start=(kc == 0),
                            stop=(kc == KC - 1),
                        )
                    nc.vector.tensor_scalar_mul(ps, ps, pad_sb[:, i : i + 1])
                    v1_i = v1_sb[:, i, :]
                    # single strided copy for all 4 head slices (the drain is
                    # overhead-dominated; one op instead of four)
                    nc.vector.tensor_copy(
                        v1_i.rearrange("p (h e) -> p h e", e=E1)[:, :, 0:D],
                        ps.rearrange("p (h e) -> p h e", e=D),
                    )
                    ones_v = v1_i.rearrange("p (h e) -> p h e", e=E1)[:, :, D : D + 1]
                    pc = pad_sb[:, i : i + 1]
                    pad_b = bass.AP(
                        tensor=pc.tensor,
                        offset=pc.offset,
                        ap=[pc.ap[0], [0, HPC], [0, 1]],
                    )
                    nc.vector.tensor_copy(ones_v, pad_b)

                def emit_y_pair(n, oc0, y_ps, last):
                    """final y chunk n, oc pair (oc0, oc0+1): jc-accumulated
                    matmuls + drain + DMA out."""
                    cols = slice(n * 512, (n + 1) * 512)
                    yo = y_sb_pool.tile(
                        [P, 2, 512], out_dt, tag="yo", name="yo_t", bufs=4
                    )
                    for u, oc in enumerate((oc0, oc0 + 1)):
                        ps = y_ps.tile([P, 512], f32, tag="y", name="y_ps_t")
                        for jc in range(2):
                            nc.tensor.matmul(
                                ps,
                                lhsT=wo_sb[:, jc, oc * P : (oc + 1) * P],
                                rhs=ctxn_sb[:, jc, cols],
                                start=(jc == 0),
                                stop=(jc == 1),
                            )
                        # while exp still runs, ACT must stay clear of drains;
                        # on the post-exp flush (last chunk) use both engines
                        eng = nc.scalar.copy if (last and u) else nc.vector.tensor_copy
                        eng(yo[:, u, :], ps)
                    out_v = yT[oc0 * P : (oc0 + 2) * P, cols].rearrange(
                        "(two p) t -> p two t", p=P
                    )
                    nc.sync.dma_start(out=out_v, in_=yo)

                # ---------- n-major attention, one fused pass-pair ----------
                from collections import deque

                fill_q = deque()  # PE filler units, popped one per iteration

                def emit_pair(h, na, fillers=(), y_stream=None):
                    """Head h, fused tq-chunk pair (na, na+1).

                    One [P,1024]-wide scores matmul + exp per tk chunk covers
                    both tq chunks (halves ACT's per-call overhead); each
                    chunk keeps its own [65,512] ctx accumulator, normalized
                    (and y-streamed) the moment it completes.
                    """
                    jc, poff = h // 2, (h % 2) * D
                    qTh = qT_sb[poff : poff + D, jc, :]
                    kTh = kT_sb[poff : poff + D, jc, :]
                    v1h = lambda i: v1_sb[:, i, h * E1 : (h + 1) * E1]
                    nb = na + 1
                    base = na * 512
                    ni = 4 * nb + 4
                    e = e_pool.tile([P, NT, 1024], av_dt, tag="e", name="e_t")
                    ctxs = [
                        ctx_ps.tile(
                            [E1, 512], f32, tag=f"ctx{c}", name="ctx_t", bufs=1
                        )
                        for c in range(2)
                    ]

                    def finish_chunk(c):
                        """normalize chunk na+c; queue its y units."""
                        n = na + c
                        cols = slice(n * 512, (n + 1) * 512)
                        rec = norm_pool.tile([1, 512], f32, tag="rec", name="rec_t")
                        bc = norm_pool.tile([D, 512], f32, tag="bc", name="bc_t")
                        nc.vector.reciprocal(rec, ctxs[c][D : D + 1, :])
                        nc.gpsimd.partition_broadcast(bc, rec)
                        nc.vector.tensor_mul(
                            ctxn_sb[poff : poff + D, jc, cols], ctxs[c][0:D, :], bc
                        )
                        if y_stream:
                            y_stream(n, fill_q)

                    def emit_av(i):
                        for c, n in ((0, na), (1, nb)):
                            if i <= 4 * n + 3:
                                nc.tensor.matmul(
                                    ctxs[c],
                                    lhsT=v1h(i),
                                    rhs=e[:, i, c * 512 : (c + 1) * 512],
                                    start=(i == 0),
                                    stop=(i == 4 * n + 3),
                                )
                        if i == 4 * na + 3:
                            finish_chunk(0)

                    fill_q.extend(fillers)
                    pending = None
                    for i in range(ni):
                        lo = max(i * P, base)  # diagonal trim
                        off = lo - base
                        s = sc_ps.tile([P, 1024], f32, tag="s", name="s_t")
                        # a matmul output cannot span two PSUM banks: split at
                        # the 512-col boundary; exp still reads the full width
                        for b0, b1 in ((off, 512), (max(off, 512), 1024)):
                            if b0 < b1:
                                nc.tensor.matmul(
                                    s[:, b0:b1],
                                    lhsT=kTh[:, i * P : (i + 1) * P],
                                    rhs=qTh[:, base + b0 : base + b1],
                                    start=True,
                                    stop=True,
                                )
                        nc.scalar.activation(
                            e[:, i, off:1024],
                            s[:, off:1024],
                            mybir.ActivationFunctionType.Exp,
                            scale=0.125,  # 1/sqrt(D)
                        )
                        nd = i // 4
                        if nd in (na, nb):  # diagonal block
                            zlo = (nd - na) * 512
                            if off > zlo:
                                nc.vector.memset(e[:, i, zlo:off], 0.0)
                            nc.vector.tensor_mul(
                                e[:, i, off : off + P],
                                e[:, i, off : off + P],
                                mask,
                            )
                        if pending is not None:
                            emit_av(pending)
                        if fill_q:
                            fill_q.popleft()()
                        pending = i
                    emit_av(pending)
                    finish_chunk(1)

                # ---------- schedule ----------
                # warmup: qk pair0 kc-outer over the full width (8 PSUM
                # banks) so every xT chunk is consumed as its DMA lands
                sc_ps = ctx_ps = None
                with tc.tile_pool(name="qk0_ps", bufs=1, space="PSUM") as qk0_ps:
                    pss = {
                        name: qk0_ps.tile(
                            [P, T], f32, tag=f"qk0{name}", name=f"ps0{name}"
                        )
                        for name in ("q", "k")
                    }
                    for kc in range(KC):
                        for name in ("q", "k"):
                            for n in range(NQ):
                                nc.tensor.matmul(
                                    pss[name][:, n * 512 : (n + 1) * 512],
                                    lhsT=w_tiles[name][:, kc, 0:P],
                                    rhs=xT_sb[:, kc, n * 512 : (n + 1) * 512],
                                    start=(kc == 0),
                                    stop=(kc == KC - 1),
                                )
                    # chunk-major drain order: pair-0 scores need (q0,k0) first
                    for n in range(NQ):
                        for name, dst in (("q", qT_sb), ("k", kT_sb)):
                            # ACT is idle pre-attention; keep DVE free for v
                            eng = nc.vector.tensor_copy if n % 2 else nc.scalar.copy
                            eng(
                                dst[:, 0, n * 512 : (n + 1) * 512],
                                pss[name][:, n * 512 : (n + 1) * 512],
                            )
                with (
                    tc.tile_pool(name="sc_ps", bufs=2, space="PSUM") as sc_ps,
                    tc.tile_pool(name="ctx_ps", bufs=2, space="PSUM") as ctx_ps,
                ):
                    with tc.tile_pool(name="v_ps", bufs=2, space="PSUM") as v_ps:
                        # v(i) streams as filler ahead of its AV use so pair-0
                        # scores (and ACT's exp) start as early as possible
                        emit_v(0, v_ps)
                        emit_v(1, v_ps)
                        vu = [lambda ii=i: emit_v(ii, v_ps) for i in range(2, 16)]
                        emit_pair(0, 0, vu[0:8])
                        emit_pair(0, 2, vu[8:14])
                    with tc.tile_pool(name="qk1_ps", bufs=2, space="PSUM") as qk1_ps:
                        # qk pair1: chunks 0,1 must finish in h1 (h2's and
                        # h3's first pairs span them); 2,3 fill h2's pair 0
                        qu = [
                            lambda nm=nm, nn=n1: emit_qk_chunk(1, nm, nn, qk1_ps)
                            for n1 in range(NQ)
                            for nm in ("q", "k")
                        ]
                        emit_pair(1, 0, qu[0:2])
                        emit_pair(1, 2, qu[2:4])
                        emit_pair(2, 0, qu[4:8])
                    with tc.tile_pool(name="y_ps", bufs=2, space="PSUM") as y_ps:

                        def y_stream(n, q):
                            last = n == NQ - 1
                            for oc0 in range(0, KC, 2):
                                q.append(
                                    lambda nn=n, oo=oc0, ll=last: emit_y_pair(
                                        nn, oo, y_ps, ll
                                    )
                                )

                        # interleave h2/h3 pairs: h3's early y units become
                        # PE filler for h2's second pair
                        emit_pair(3, 0, (), y_stream)
                        emit_pair(2, 2)
                        emit_pair(3, 2, (), y_stream)
                        while fill_q:
                            fill_q.popleft()()

    nc.compile()
    return nc


def make_in_maps(x, pad_mask, Wq, Wk, Wv, Wo):
    """Host-side sharding: per-core input dict."""
    if MM_DTYPE == "bfloat16":
        import ml_dtypes

        in_np = ml_dtypes.bfloat16
    else:
        in_np = np.float32
    x = np.asarray(x, dtype=np.float32)
    pad_f = np.asarray(pad_mask).astype(np.float32).reshape(B, T, 1)
    # cast once, slice per core (cheaper than per-core casting)
    xT_b = [np.ascontiguousarray(x[b].T).astype(in_np) for b in range(B)]
    WqT, WkT, WvT = (
        np.asarray(w, dtype=np.float32).T.astype(in_np) for w in (Wq, Wk, Wv)
    )
    Wo_c = np.asarray(Wo, dtype=np.float32).astype(in_np)
    in_maps = []
    for c in range(NCORES):
        b, g = c // GROUPS, c % GROUPS
        jr = slice(g * J, (g + 1) * J)
        in_maps.append(
            {
                "xT": xT_b[b],
                "wq_t": np.ascontiguousarray(WqT[:, jr]),
                "wk_t": np.ascontiguousarray(WkT[:, jr]),
                "wv_t": np.ascontiguousarray(WvT[:, jr]),
                "wo_t": np.ascontiguousarray(Wo_c[:, jr].T),
                "pad": np.ascontiguousarray(pad_f[b]),
            }
        )
    return in_maps


def unshard(results):
    """Sum the 4 tensor-parallel partials per batch; transpose back."""
    y = np.empty((B, T, C), dtype=np.float32)
    for b in range(B):
        acc = results[b * GROUPS]["yT"].astype(np.float32)
        for g in range(1, GROUPS):
            acc = acc + results[b * GROUPS + g]["yT"].astype(np.float32)
        y[b] = acc.T
    return y


def kernel(x, pad_mask, Wq, Wk, Wv, Wo):
    global _COMPILED
    from concourse.bass_utils import run_bass_kernel_spmd

    if _COMPILED is None:
        _COMPILED = build_program()
    in_maps = make_in_maps(x, pad_mask, Wq, Wk, Wv, Wo)
    res = run_bass_kernel_spmd(_COMPILED, in_maps, core_ids=list(range(NCORES)))
    return unshard(res.results)
